# revision 5
# baseline (speedup 1.0000x reference)
"""Causal multi-head self-attention with RoPE on 8 Trainium2 NeuronCores.

Problem: B=2, S=2048, D=2048, 16 heads x head_dim 128, causal mask, RoPE.

Sharding (data + tensor parallel per the hint): 8 cores = 2 batch rows x 4
head-groups (4 heads each). Each core computes, for its batch row and its 4
heads: Q/K/V projections, RoPE, causal softmax attention, and the partial
output projection through its head-group's slice of Wo. The host sums the 4
head-group partials per batch row (row-parallel Wo unshard).

v4 (l-chain elimination + causal trimming + DMA phasing), on top of the
bf16 + software-pipelined v3:

  - The softmax denominator no longer costs a PE pass over the weights.
    The O-accumulation matmul is reoriented to out[q,hd]: stationary = a
    128-wide q-chunk of the exp'd scores w [k,q], moving = [V | 1] (the V
    tile padded with a ones column), so column 128 of the PSUM accumulates
    sum_k w[k,q] for free. 1/l is then a per-partition scalar multiply
    (native DVE broadcast), and O is transposed back to [hd,q] for the Wo
    matmul with a cheap 128-col PE transpose.
  - Causal trimming: diagonal score tiles only compute the surviving
    column range, and O-chunk matmuls that are fully above the diagonal
    are skipped.
  - Projections for quarters >= 1 run 2 heads per pass (two passes) so
    their PSUM footprint is 2 banks; attention + everything else shares a
    6-slot PSUM ring. Quarter 0 runs 4-wide out of the shared ring (the
    ring is otherwise idle) so its DMA stream stays ahead of the PE.
  - Weight DMAs are issued in first-use order (wk with the k-phase, wq
    with q, wv with v), RoPE tables are loaded in per-quarter slices, and
    Wo streams in during quarter-0 attention.
  - Quarter 2's Wo matmuls are deferred into the tail (quarter 3 has no
    projections to interleave) to cover the exp-latency exposure there.
  - RoPE runs all-bf16 on the DVE; the PSUM->SBUF projection copy moved
    to the ACT engine.
"""

import math

import numpy as np

import concourse.bass as bass
import concourse.mybir as mybir
import concourse.tile as tile
from concourse import bacc

B = 2
D = 2048
H_LOC = 4  # heads per core
HD = 128  # head dim
QW = 512  # s-quarter width (and matmul moving width)
N_CORES = 8
THETA = 10000.0
F32 = mybir.dt.float32
BF16 = mybir.dt.bfloat16


def build_program(S=2048, repeat=1):
    """Build the per-core SPMD Bass program (all 8 cores run this).

    repeat>1 re-runs the whole computation serially inside one NEFF;
    used only to measure on-device execution time via the wall-time slope.
    """
    nq = S // QW  # number of s-quarters
    dc = D // HD  # contraction chunks
    scale = 1.0 / math.sqrt(HD)

    nc = bacc.Bacc("TRN2", target_bir_lowering=False, debug=False, num_devices=N_CORES)
    xt_d = nc.dram_tensor("xt", [D, S], BF16, kind="ExternalInput").ap()
    wqt_d = nc.dram_tensor("wqt", [D, H_LOC * HD], BF16, kind="ExternalInput").ap()
    wkt_d = nc.dram_tensor("wkt", [D, H_LOC * HD], BF16, kind="ExternalInput").ap()
    wvt_d = nc.dram_tensor("wvt", [D, H_LOC * HD], BF16, kind="ExternalInput").ap()
    wot_d = nc.dram_tensor("wot", [H_LOC * HD, D], BF16, kind="ExternalInput").ap()
    cos2_d = nc.dram_tensor("cos2", [HD, S], BF16, kind="ExternalInput").ap()
    sinpm_d = nc.dram_tensor("sinpm", [HD, S], BF16, kind="ExternalInput").ap()
    ident_d = nc.dram_tensor("ident", [HD, HD], BF16, kind="ExternalInput").ap()
    outt_d = nc.dram_tensor("outt", [D, S], BF16, kind="ExternalOutput").ap()

    with tile.TileContext(nc) as tc:
        with (
            tc.tile_pool(name="const", bufs=1) as constp,
            tc.tile_pool(name="weights", bufs=1) as wp,
            tc.tile_pool(name="ktv", bufs=1) as ktvp,
            tc.tile_pool(name="xtp", bufs=2) as xtp,
            tc.tile_pool(name="qtp", bufs=2) as qtp,
            tc.tile_pool(name="rope", bufs=2) as rtp,
            tc.tile_pool(name="wexp", bufs=18) as wep,
            tc.tile_pool(name="otT", bufs=2) as otTp,
            tc.tile_pool(name="norm", bufs=3) as nrmp,
            tc.tile_pool(name="outsb", bufs=3) as outsbp,
            tc.tile_pool(name="ps", bufs=6, space="PSUM") as psp,
        ):
            # Resident weights (bf16, loaded once at first use):
            # wq/wk/wv: [128, dc*QW] where block d holds W[d-chunk, :512].
            # wo: [128, 4*D] where block h holds Wo rows of head h.
            wk_sb = wp.tile([HD, dc * QW], BF16, tag="wk", name="wk_sb")
            wq_sb = wp.tile([HD, dc * QW], BF16, tag="wq", name="wq_sb")
            wv_sb = wp.tile([HD, dc * QW], BF16, tag="wv", name="wv_sb")
            wo_sb = wp.tile([HD, 4 * D], BF16, tag="wo", name="wo_sb")

            # per-quarter RoPE table slices (bf16)
            cosq = [
                constp.tile([HD, QW], BF16, tag=f"cs{q}", name=f"cs{q}")
                for q in range(nq)
            ]
            sinq = [
                constp.tile([HD, QW], BF16, tag=f"sn{q}", name=f"sn{q}")
                for q in range(nq)
            ]
            ident = constp.tile([HD, HD], BF16, tag="ident", name="ident_sb")
            loaded = {"w": False, "wo": False, "ident": False}
            tabs_loaded = [False] * nq

            kt = [
                ktvp.tile([HD, S], BF16, tag=f"kt{h}", name=f"kt{h}")
                for h in range(H_LOC)
            ]
            # V tiles: [k=128, head, 129] with col 128 of each head = ones
            # (the ones column makes the O-matmul accumulate the softmax
            # denominator in PSUM column 128).
            vt = [
                ktvp.tile([HD, H_LOC, HD + 1], BF16, tag=f"v{i}", name=f"v{i}")
                for i in range(S // HD)
            ]

            # pair-partner swap: +-16 within each 32-partition quadrant
            SHUF_MASK = [(i + 16) % 32 for i in range(32)]

            def rope(ps, q, out_ap):
                # out = R(pos) * ps, lane-local thanks to the host-side
                # head-dim permutation that places each RoPE pair partner 16
                # partitions away within the same 32-lane quadrant, so the
                # cross-partition move is a single DVE stream_shuffle.
                # sinpm carries the pair sign (-sin even slot, +sin odd slot).
                # The PSUM->SBUF downconvert runs on the ACT engine; the
                # rotation itself is all-bf16 on the DVE.
                rs = rtp.tile([HD, QW], BF16, tag="rs", name="rs", bufs=2)
                nc.scalar.copy(rs[:], ps[:])
                shuf = rtp.tile([HD, QW], BF16, tag="shuf", name="shuf", bufs=2)
                nc.vector.stream_shuffle(shuf[:], rs[:], SHUF_MASK)
                ta = rtp.tile([HD, QW], BF16, tag="ta", name="ta", bufs=2)
                nc.vector.tensor_mul(ta[:], rs[:], cosq[q][:])
                tb = rtp.tile([HD, QW], BF16, tag="tb", name="tb", bufs=2)
                nc.vector.tensor_mul(tb[:], shuf[:], sinq[q][:])
                nc.vector.tensor_add(out_ap, ta[:], tb[:])

            qts_store = {}

            def proj_granules(q, wide):
                """Projections for quarter q as (pe_cost_ns, emit_fn) list.

                wide=True: 4 heads per pass (4 PSUM slots from the shared
                ring) -- used for quarter 0 where the ring is idle and the
                DMA stream must stay ahead. wide=False: 2 heads per pass,
                two passes, 2 PSUM slots ("pp" tag).
                """
                sl = slice(q * QW, (q + 1) * QW)
                xt = [None] * dc
                pps = {}
                gran = []
                tag = "pa" if wide else "pp"
                hs_groups = [range(H_LOC)] if wide else [(0, 1), (2, 3)]

                def k_chunk(d, hs, first, q=q, sl=sl):
                    def f():
                        if d == 0:
                            pps['k'] = {
                                h: psp.tile(
                                    [HD, QW], F32, tag=tag, name=f"kps{h}",
                                    bufs=6 if wide else 2,
                                )
                                for h in hs
                            }
                        if first:
                            x_t = xtp.tile([HD, QW], BF16, tag=f"x{d}", name=f"x{d}")
                            nc.sync.dma_start(
                                x_t[:], xt_d[d * HD : (d + 1) * HD, sl]
                            )
                            xt[d] = x_t
                            if not loaded["w"]:
                                nc.sync.dma_start(
                                    wk_sb[:, d * QW : (d + 1) * QW],
                                    wkt_d[d * HD : (d + 1) * HD, :],
                                )
                            if d == 2 and not tabs_loaded[q]:
                                tabs_loaded[q] = True
                                nc.sync.dma_start(cosq[q][:], cos2_d[:, sl])
                                nc.sync.dma_start(sinq[q][:], sinpm_d[:, sl])
                        for h in hs:
                            nc.tensor.matmul(
                                pps['k'][h][:],
                                wk_sb[:, d * QW + h * HD : d * QW + (h + 1) * HD],
                                xt[d][:],
                                start=(d == 0),
                                stop=(d == dc - 1),
                            )
                    return f

                def rope_k(h, q=q):
                    def f():
                        rope(pps['k'][h][:], q, kt[h][:, q * QW : (q + 1) * QW])
                    return f

                def q_chunk(d, hs, first, q=q):
                    def f():
                        if d == 0:
                            pps['q'] = {
                                h: psp.tile(
                                    [HD, QW], F32, tag=tag, name=f"qps{h}",
                                    bufs=6 if wide else 2,
                                )
                                for h in hs
                            }
                        if first and not loaded["w"]:
                            nc.sync.dma_start(
                                wq_sb[:, d * QW : (d + 1) * QW],
                                wqt_d[d * HD : (d + 1) * HD, :],
                            )
                        for h in hs:
                            nc.tensor.matmul(
                                pps['q'][h][:],
                                wq_sb[:, d * QW + h * HD : d * QW + (h + 1) * HD],
                                xt[d][:],
                                start=(d == 0),
                                stop=(d == dc - 1),
                            )
                    return f

                def rope_q(h, q=q):
                    def f():
                        q_sb = qtp.tile([HD, QW], BF16, tag=f"qt{h}", name=f"qt{h}")
                        rope(pps['q'][h][:], q, q_sb[:])
                        qts_store[q][h] = q_sb
                    return f

                def v_chunk(d, sts, first, q=q):
                    def f():
                        if d == 0:
                            pps['v'] = {
                                st: psp.tile(
                                    [HD, H_LOC, HD], F32, tag=tag, name=f"vps{st}",
                                    bufs=6 if wide else 2,
                                )
                                for st in sts
                            }
                        if first and not loaded["w"]:
                            nc.sync.dma_start(
                                wv_sb[:, d * QW : (d + 1) * QW],
                                wvt_d[d * HD : (d + 1) * HD, :],
                            )
                            if d == dc - 1:
                                loaded["w"] = True
                        for st in sts:
                            nc.tensor.matmul(
                                pps['v'][st][:, :, :],
                                xt[d][:, st * HD : (st + 1) * HD],
                                wv_sb[:, d * QW : (d + 1) * QW],
                                start=(d == 0),
                                stop=(d == dc - 1),
                            )
                    return f

                def v_copy(st, q=q):
                    def f():
                        # scatter the 4 heads' V into the ones-padded layout
                        v = vt[q * 4 + st]
                        nc.vector.tensor_copy(v[:, :, 0:HD], pps['v'][st][:, :, :])
                        nc.vector.memset(v[:, :, HD : HD + 1], 1.0)
                    return f

                qts_store[q] = [None] * H_LOC
                mm = 853 if wide else 427
                for gi, hs in enumerate(hs_groups):
                    for d in range(dc):
                        gran.append((mm, k_chunk(d, hs, first=(gi == 0))))
                    for h in hs:
                        gran.append((30, rope_k(h)))
                for gi, hs in enumerate(hs_groups):
                    for d in range(dc):
                        gran.append((mm, q_chunk(d, hs, first=(gi == 0))))
                    for h in hs:
                        gran.append((30, rope_q(h)))
                for gi, sts in enumerate(hs_groups):
                    for d in range(dc):
                        gran.append((mm, v_chunk(d, sts, first=(gi == 0))))
                    for st in sts:
                        gran.append((20, v_copy(st)))
                return gran

            def attn_granules(q):
                """Attention for quarter q as (pe_cost_ns, emit_fn) list."""
                nk = (q + 1) * 4
                state = {'wts': {}, 'otT': [None] * H_LOC}
                gran = []

                def score_g(h, ki, q=q, nk=nk):
                    # scores^T tile [k=128, q-range] -> exp -> w (bf16 SBUF);
                    # diagonal tiles only compute surviving columns.
                    diag = ki >= 4 * q
                    off = (ki - 4 * q) * HD if diag else 0
                    def f():
                        if loaded["ident"] is False:
                            loaded["ident"] = True
                            nc.sync.dma_start(ident[:], ident_d[:])
                            for hb in range(H_LOC):
                                nc.sync.dma_start(
                                    wo_sb[:, hb * D : (hb + 1) * D],
                                    wot_d[hb * HD : (hb + 1) * HD, :],
                                )
                        s_ps = psp.tile([HD, QW], F32, tag="pa", name="s_ps")
                        nc.tensor.matmul(
                            s_ps[:, off:],
                            kt[h][:, ki * HD : (ki + 1) * HD],
                            qts_store[q][h][:, off:],
                            start=True,
                            stop=True,
                        )
                        w_t = wep.tile([HD, QW], BF16, tag="wexp", name="w_t")
                        nc.scalar.activation(
                            w_t[:, off:],
                            s_ps[:, off:],
                            mybir.ActivationFunctionType.Exp,
                            scale=scale,
                        )
                        if diag:
                            nc.gpsimd.affine_select(
                                out=w_t[:, off:],
                                in_=w_t[:, off:],
                                compare_op=mybir.AluOpType.is_ge,
                                fill=0.0,
                                base=q * QW - ki * HD + off,
                                pattern=[[1, QW - off]],
                                channel_multiplier=-1,
                            )
                        state['wts'][ki] = w_t
                    return f

                def og(h, c, q=q, nk=nk):
                    # O'[q-chunk, 0:128] = sum_k V^T w ; O'[:, 128] = sum_k w
                    # (the l column). Stationary = w q-chunk, moving = [V|1].
                    cg = 4 * q + c
                    cnt = min(nk, cg + 1)
                    def f():
                        ot = psp.tile([HD, HD + 1], F32, tag="pa", name="ot")
                        state['ot', c] = ot
                        for ki in range(cnt):
                            nc.tensor.matmul(
                                ot[:],
                                state['wts'][ki][:, c * HD : (c + 1) * HD],
                                vt[ki][:, h : h + 1, :],
                                start=(ki == 0),
                                stop=(ki == cnt - 1),
                            )
                    return f

                def norm_dve(h, c, q=q):
                    # 1/l is a per-partition scalar in the [q,hd] layout:
                    # native free-axis broadcast on the DVE, no
                    # partition_broadcast needed.
                    def f():
                        ot = state['ot', c]
                        rc = nrmp.tile([HD, 1], F32, tag="rc", name="rc")
                        nc.vector.reciprocal(rc[:], ot[:, HD : HD + 1])
                        o_sb = nrmp.tile([HD, HD], BF16, tag="osb", name="o_sb")
                        nc.vector.tensor_scalar_mul(o_sb[:], ot[:, 0:HD], rc[:])
                        state['osb', c] = o_sb
                    return f

                def norm_tr(h, c, q=q):
                    # transpose O[q,hd] -> O^T[hd,q] for the Wo matmul
                    def f():
                        otT_ps = psp.tile([HD, HD], BF16, tag="pa", name="otT_ps")
                        nc.tensor.transpose(otT_ps[:], state['osb', c][:], ident[:])
                        if c == 0:
                            state['otT'][h] = otTp.tile(
                                [HD, QW], BF16, tag=f"otT{h}", name=f"otT{h}"
                            )
                        nc.vector.tensor_copy(
                            state['otT'][h][:, c * HD : (c + 1) * HD], otT_ps[:]
                        )
                    return f

                for h in range(H_LOC):
                    for ki in range(nk):
                        diag = ki >= 4 * q
                        w = QW - ((ki - 4 * q) * HD if diag else 0)
                        gran.append((int(213 * w / QW), score_g(h, ki)))
                    # stagger: the PE transpose for chunk c is emitted two
                    # granules after its DVE norm so the reciprocal+scale
                    # chain latency hides behind other PE work.
                    for c in range(H_LOC):
                        cnt = min(nk, 4 * q + c + 1)
                        gran.append((54 * cnt, og(h, c)))
                        gran.append((1, norm_dve(h, c)))
                        if c >= 1:
                            gran.append((60, norm_tr(h, c - 1)))
                    gran.append((60, norm_tr(h, H_LOC - 1)))
                return gran, state

            def wo_granules(q, attn_state):
                """Wo for quarter q: out^T[d,q] += Wo_h^T O_h^T, 2 d-subtiles
                per PSUM group (2 shared-ring slots)."""
                sl = slice(q * QW, (q + 1) * QW)
                gran = []
                wst = {}

                def wo_g(g, h, q=q):
                    def f():
                        if h == 0:
                            wst[g] = [
                                psp.tile([HD, QW], F32, tag="pa", name=f"ops{dt}")
                                for dt in range(2)
                            ]
                        for dt in range(2):
                            nc.tensor.matmul(
                                wst[g][dt][:],
                                wo_sb[
                                    :,
                                    h * D + g * 2 * HD + dt * HD
                                    : h * D + g * 2 * HD + (dt + 1) * HD,
                                ],
                                attn_state['otT'][h][:],
                                start=(h == 0),
                                stop=(h == H_LOC - 1),
                            )
                    return f

                def wo_out(g, q=q, sl=sl):
                    def f():
                        for dt in range(2):
                            rg = g * 2 + dt
                            o_sb = outsbp.tile(
                                [HD, QW], BF16, tag="osb", name="o_sb"
                            )
                            nc.vector.tensor_copy(o_sb[:], wst[g][dt][:])
                            nc.sync.dma_start(
                                outt_d[rg * HD : (rg + 1) * HD, sl], o_sb[:]
                            )
                    return f

                for g in range(8):
                    for h in range(H_LOC):
                        gran.append((427, wo_g(g, h)))
                    gran.append((40, wo_out(g)))
                return gran

            def merge(a, b):
                """Interleave two granule lists by cumulative-cost ratio."""
                ca = sum(c for c, _ in a) or 1
                cb = sum(c for c, _ in b) or 1
                ia = ib = 0
                ra = rb = 0.0
                while ia < len(a) or ib < len(b):
                    if ib >= len(b) or (
                        ia < len(a)
                        and (ra + a[ia][0]) / ca <= (rb + b[ib][0]) / cb
                    ):
                        ra += a[ia][0]
                        a[ia][1]()
                        ia += 1
                    else:
                        rb += b[ib][0]
                        b[ib][1]()
                        ib += 1

            for r in range(repeat):
                # prologue: quarter-0 projections; first rep runs them alone
                # (wide, DMA-paced), later reps merge them into the previous
                # rep's tail.
                if r == 0:
                    for _, f in proj_granules(0, wide=True):
                        f()
                att = {}
                att[0], st0 = attn_granules(0)
                carry = wo_granules(0, st0)  # wo(0) appended to attn(0)
                merge(att[0] + carry, proj_granules(1, wide=False))
                att[1], st1 = attn_granules(1)
                merge(att[1] + wo_granules(1, st1), proj_granules(2, wide=False))
                att[2], st2 = attn_granules(2)
                merge(att[2], proj_granules(3, wide=False))
                wo2 = wo_granules(2, st2)
                att[3], st3 = attn_granules(3)
                if r + 1 < repeat:
                    merge(att[3] + wo2, proj_granules(0, wide=False))
                else:
                    merge(att[3], wo2)
                for _, f in wo_granules(3, st3):
                    f()
    nc.compile()
    return nc


def prep_inputs(x, token_positions, Wq, Wk, Wv, Wo):
    """Shard + lay out the full inputs into 8 per-core input maps."""
    import ml_dtypes

    bf16 = ml_dtypes.bfloat16
    S = x.shape[1]
    x = np.asarray(x, np.float32)
    pos = np.asarray(token_positions).astype(np.float32)
    k = np.arange(HD // 2, dtype=np.float32)
    inv_freq = (1.0 / (THETA ** (2.0 * k / HD))).astype(np.float32)
    freqs = pos[:, None] * inv_freq[None, :]  # [S, 64]
    cos = np.cos(freqs).T.astype(np.float32)  # [64, S]
    sin = np.sin(freqs).T.astype(np.float32)
    # head-dim permutation chosen so each RoPE pair partner sits +-16
    # partitions away within the same 32-partition quadrant (enables the
    # on-device stream_shuffle). Partition n holds:
    #   g, r = divmod(n, 32); j = 16*g + (r % 16)   (frequency index)
    #   original dim 2j   if r < 16 ("even" slot, rotates with -sin)
    #   original dim 2j+1 otherwise ("odd" slot, rotates with +sin)
    n = np.arange(HD)
    g, r = n // 32, n % 32
    j = 16 * g + (r % 16)
    odd = (r >= 16).astype(np.int64)
    perm = 2 * j + odd
    cos2 = np.ascontiguousarray(cos[j]).astype(bf16)  # [128, S]
    sinpm = np.ascontiguousarray(np.where(odd[:, None], sin[j], -sin[j])).astype(
        bf16
    )
    ident = np.eye(HD, dtype=bf16)
    xts = [np.ascontiguousarray(x[b].T).astype(bf16) for b in range(B)]

    in_maps = []
    for c in range(N_CORES):
        b, hg = c // 4, c % 4
        rows = slice(hg * H_LOC * HD, (hg + 1) * H_LOC * HD)

        def permT(W):
            Wg = np.asarray(W, np.float32)[rows]  # [512, D]
            Wg = Wg.reshape(H_LOC, HD, D)[:, perm, :].reshape(H_LOC * HD, D)
            return np.ascontiguousarray(Wg.T).astype(bf16)  # [D, 512]

        in_maps.append(
            {
                "xt": xts[b],
                "wqt": permT(Wq),
                "wkt": permT(Wk),
                "wvt": np.ascontiguousarray(
                    np.asarray(Wv, np.float32)[rows].T
                ).astype(bf16),
                "wot": np.ascontiguousarray(
                    np.asarray(Wo, np.float32)[:, rows].T
                ).astype(bf16),
                "cos2": cos2,
                "sinpm": sinpm,
                "ident": ident,
            }
        )
    return in_maps


def combine_outputs(outts):
    """outts: list of 8 per-core outT [D, S] partials -> full [B, S, D]."""
    return np.stack(
        [
            sum(np.asarray(o, np.float32) for o in outts[b * 4 : (b + 1) * 4])
            .T.astype(np.float32)
            for b in range(B)
        ]
    )


_NC = None


def _get_nc():
    global _NC
    if _NC is None:
        _NC = build_program()
    return _NC


def kernel(x, token_positions, Wq, Wk, Wv, Wo):
    from concourse.bass_utils import run_bass_kernel_spmd

    nc = _get_nc()
    in_maps = prep_inputs(x, token_positions, Wq, Wk, Wv, Wo)
    res = run_bass_kernel_spmd(nc, in_maps, core_ids=list(range(N_CORES)))
    return combine_outputs([r["outt"] for r in res.results])
